# revision 12
# baseline (speedup 1.0000x reference)
"""Brevitas 4-bit quantized linear layer on 8 TRN2 NeuronCores.

y = x @ dequant(w)^T + dequant(bias), with per-output-channel symmetric
abs-max scales (narrow 4-bit range [-7, 7], round-half-even).

Sharding: data-parallel over tokens. x [4,2048,4096] flattens to
[8192, 4096]; each core gets 1024 rows plus the full weight + bias and
produces its 1024 rows of the output. Host concatenates the 8 shards.

Host prep (layout + per-channel metadata, all exact f32 to match the
reference bit-for-bit):
  - x shard pre-transposed to [in=4096, tok=1024] and pre-cast to bf16 so
    the kernel DMAs k-major tiles directly (PE contracts along partitions).
  - w pre-transposed to wT [in, out] f32 (pure layout; values untouched).
  - per-out-channel scale = max(absmax(|w|,axis=in), 2e-16)/7, its
    reciprocal, and the dequantized bias row (4096 floats of metadata, as
    in the sharding hint's "weight and its per-channel scales + bias").

Per-core kernel (single TileContext, Tile handles sync/overlap):
  1. xT tiles [128,1024] DMA'd into a resident xT buffer; scale/inv rows
     broadcast across partitions into [128, 4096] tiles (step-0 DMA).
  2. wT streamed in [128, 4x512] tile groups; quantize IN PLACE in the
     transposed layout (scale varies along the free=out axis):
       DVE  s  = s * inv_bcast           (tensor_tensor)
       ACT  s  = Copy(s + 1.5*2^23)      (magic round-half-even)
       DVE  wq = (s - 1.5*2^23) * scale_bcast -> bf16  (scalar_tensor_tensor)
     writing straight into the w_deqT chunk buffer [4096, 512]. No device
     transposes at all.
  3. Per chunk: 8 psum tiles [tok=128, out=512], each initialized with a
     rank-1 bias matmul (ones[1,128]^T @ b_deq_row[1,512]) then 32
     accumulating matmuls over k; ACT-evicted to SBUF and DMA'd out.
"""
import os
import numpy as np
import ml_dtypes

import concourse.bass as bass
import concourse.mybir as mybir
import concourse.tile as tile
from concourse import bacc
from concourse.bass_utils import run_bass_kernel_spmd

P = 128
K = 4096            # in_features
OUT = 4096          # out_features
TOK = 1024          # tokens per core (8192 / 8 cores)
N_CORES = 8
CHUNK = 512         # out-features per matmul chunk
GRP = 4             # k-tiles quantized per pass
MAGIC = float(np.float32(1.5 * 2**23))
KT = K // P         # 32 k-tiles
MT = TOK // P       # 8 token tiles
NCHUNK = OUT // CHUNK  # 8 chunks

_cache = {}


def _build(mmdt):
    f32 = mybir.dt.float32
    nc = bacc.Bacc(None, target_bir_lowering=False)
    x_in = nc.declare_dram_parameter("x", [K, TOK], mmdt, isOutput=False)
    wT_in = nc.declare_dram_parameter("wT", [K, OUT], f32, isOutput=False)
    scale_in = nc.declare_dram_parameter("scale_row", [OUT], f32, isOutput=False)
    inv_in = nc.declare_dram_parameter("inv_row", [OUT], f32, isOutput=False)
    bdeq_in = nc.declare_dram_parameter("bdeq_row", [OUT], f32, isOutput=False)
    y_out = nc.declare_dram_parameter("y", [TOK, OUT], f32, isOutput=True)

    with tile.TileContext(nc) as tc:
        with tc.tile_pool(name="const", bufs=1) as const, \
             tc.tile_pool(name="xTp", bufs=1) as xTp, \
             tc.tile_pool(name="wTp", bufs=2) as wTp, \
             tc.tile_pool(name="stage", bufs=3) as stage, \
             tc.tile_pool(name="outp", bufs=4) as outp, \
             tc.tile_pool(name="mmps", bufs=6, space="PSUM") as mmps:

            ones_row = const.tile([1, P], mmdt)
            nc.vector.memset(ones_row[:], 1.0)

            brow = const.tile([1, OUT], mmdt)
            # SWDGE casts f32 -> bf16 during the transfer
            nc.gpsimd.dma_start(out=brow[:], in_=bdeq_in[:])

            def bcast_row(dram_param):
                a = dram_param[:]
                return bass.AP(tensor=a.tensor, offset=a.offset,
                               ap=[[0, P]] + list(a.ap))

            scale_bc = const.tile([P, OUT], f32)
            nc.gpsimd.dma_start(out=scale_bc[:], in_=bcast_row(scale_in))
            inv_bc = const.tile([P, OUT], f32)
            nc.gpsimd.dma_start(out=inv_bc[:], in_=bcast_row(inv_in))

            # ---- resident pre-transposed x ----
            xT = xTp.tile([P, KT * TOK], mmdt, name="xT")  # [:, kt*TOK + t]
            xT3 = xT[:].rearrange("p (kt t) -> p kt t", kt=KT)
            for kt in range(KT):
                nc.sync.dma_start(
                    out=xT3[:, kt, :], in_=x_in[kt * P:(kt + 1) * P, :])

            wT3d = wT_in[:].rearrange("(kt p) o -> p kt o", p=P)

            wTc3s = {}

            def quant_chunk(c):
                csl = slice(c * CHUNK, (c + 1) * CHUNK)
                wTc = wTp.tile([P, KT * CHUNK], mmdt, tag="wT")
                wTc3 = wTc[:].rearrange("p (kt t) -> p kt t", kt=KT)
                wTc3s[c] = wTc3
                for g in range(KT // GRP):
                    s = stage.tile([P, GRP * CHUNK], f32, tag="stage")
                    s3 = s[:].rearrange("p (i t) -> p i t", i=GRP)
                    nc.sync.dma_start(
                        out=s3, in_=wT3d[:, g * GRP:(g + 1) * GRP, csl])
                    inv_b = inv_bc[:, csl].unsqueeze(1).broadcast_to(
                        (P, GRP, CHUNK))
                    sc_b = scale_bc[:, csl].unsqueeze(1).broadcast_to(
                        (P, GRP, CHUNK))
                    nc.vector.tensor_tensor(
                        out=s3, in0=s3, in1=inv_b, op=mybir.AluOpType.mult)
                    nc.scalar.activation(
                        s3, s3, mybir.ActivationFunctionType.Copy, bias=MAGIC)
                    nc.vector.scalar_tensor_tensor(
                        out=wTc3[:, g * GRP:(g + 1) * GRP, :], in0=s3,
                        scalar=MAGIC, in1=sc_b,
                        op0=mybir.AluOpType.subtract, op1=mybir.AluOpType.mult)

            def matmul_chunk(c):
                csl = slice(c * CHUNK, (c + 1) * CHUNK)
                wTc3 = wTc3s.pop(c)
                for mt in range(MT):
                    ps = mmps.tile([P, CHUNK], f32, tag="mm")
                    nc.tensor.matmul(
                        ps[:], ones_row[:], brow[:, csl], start=True, stop=False)
                    for kt in range(KT):
                        nc.tensor.matmul(
                            ps[:],
                            xT3[:, kt, mt * P:(mt + 1) * P],
                            wTc3[:, kt, :],
                            start=False, stop=(kt == KT - 1))
                    ysb = outp.tile([P, CHUNK], f32, tag="ysb")
                    nc.scalar.activation(
                        ysb[:], ps[:], mybir.ActivationFunctionType.Copy)
                    nc.sync.dma_start(
                        out=y_out[mt * P:(mt + 1) * P, csl], in_=ysb[:])

            for c in range(NCHUNK + 1):
                if c < NCHUNK:
                    quant_chunk(c)
                if c >= 1:
                    matmul_chunk(c - 1)
    nc.compile()
    return nc


def _get_nc(mmdt):
    key = str(mmdt)
    if key not in _cache:
        _cache[key] = _build(mmdt)
    return _cache[key]


def _host_prep(x, weight, bias_param):
    B, S, _K = x.shape
    xb = np.asarray(x, dtype=np.float32).reshape(B * S, K).astype(ml_dtypes.bfloat16)
    w = np.asarray(weight, dtype=np.float32)
    b = np.asarray(bias_param, dtype=np.float32)

    # exact-f32 per-channel quant metadata (matches the jax reference ops)
    absmax = np.max(np.abs(w), axis=1)
    scale = (np.maximum(absmax, np.float32(2e-16)) / np.float32(7.0)).astype(np.float32)
    inv = (np.float32(1.0) / scale).astype(np.float32)
    bdeq = (np.round(b / scale) * scale).astype(np.float32)

    wT = np.ascontiguousarray(w.T)
    shards = [np.ascontiguousarray(xb[i * TOK:(i + 1) * TOK].T)
              for i in range(N_CORES)]
    return shards, wT, scale, inv, bdeq


def kernel(x: np.ndarray, weight: np.ndarray, bias_param: np.ndarray) -> np.ndarray:
    B, S, _K = x.shape
    assert (B * S, _K) == (TOK * N_CORES, K), (x.shape,)
    nc = _get_nc(mybir.dt.bfloat16)

    shards, wT, scale, inv, bdeq = _host_prep(x, weight, bias_param)
    in_maps = [
        {"x": shards[i], "wT": wT, "scale_row": scale,
         "inv_row": inv, "bdeq_row": bdeq}
        for i in range(N_CORES)
    ]
    trace = os.environ.get("BRW_TRACE", "0") == "1"
    res = run_bass_kernel_spmd(
        nc, in_maps, core_ids=list(range(N_CORES)), trace=trace)
    if trace:
        print(f"HW exec time: {res.exec_time_ns} ns", flush=True)
        kernel.last_exec_time_ns = res.exec_time_ns
        kernel.last_trace = res.instructions_and_trace
    y = np.concatenate([res.results[i]["y"] for i in range(N_CORES)], axis=0)
    return y.reshape(B, S, OUT)


# revision 13
# speedup vs baseline: 1.0153x; 1.0153x over previous
"""Brevitas 4-bit quantized linear layer on 8 TRN2 NeuronCores.

y = x @ dequant(w)^T + dequant(bias), with per-output-channel symmetric
abs-max scales (narrow 4-bit range [-7, 7], round-half-even).

Sharding: data-parallel over tokens. x [4,2048,4096] flattens to
[8192, 4096]; each core gets 1024 rows plus the full weight + bias and
produces its 1024 rows of the output. Host concatenates the 8 shards.

Host prep (layout + per-channel metadata, all exact f32 to match the
reference bit-for-bit):
  - x shard pre-transposed to [in=4096, tok=1024] and pre-cast to bf16 so
    the kernel DMAs k-major tiles directly (PE contracts along partitions).
  - w pre-transposed to wT [in, out] f32 (pure layout; values untouched).
  - per-out-channel scale = max(absmax(|w|,axis=in), 2e-16)/7, its
    reciprocal, and the dequantized bias row (4096 floats of metadata, as
    in the sharding hint's "weight and its per-channel scales + bias").

Per-core kernel (single TileContext, Tile handles sync/overlap):
  1. xT tiles [128,1024] DMA'd into a resident xT buffer; scale/inv rows
     broadcast across partitions into [128, 4096] tiles (step-0 DMA).
  2. wT streamed in [128, 4x512] tile groups; quantize IN PLACE in the
     transposed layout (scale varies along the free=out axis):
       DVE  s  = s * inv_bcast           (tensor_tensor)
       ACT  s  = Copy(s + 1.5*2^23)      (magic round-half-even)
       DVE  wq = (s - 1.5*2^23) * scale_bcast -> bf16  (scalar_tensor_tensor)
     writing straight into the w_deqT chunk buffer [4096, 512]. No device
     transposes at all.
  3. Per chunk: 8 psum tiles [tok=128, out=512], each initialized with a
     rank-1 bias matmul (ones[1,128]^T @ b_deq_row[1,512]) then 32
     accumulating matmuls over k; ACT-evicted to SBUF and DMA'd out.
"""
import os
import numpy as np
import ml_dtypes

import concourse.bass as bass
import concourse.mybir as mybir
import concourse.tile as tile
from concourse import bacc
from concourse.bass_utils import run_bass_kernel_spmd

P = 128
K = 4096            # in_features
OUT = 4096          # out_features
TOK = 1024          # tokens per core (8192 / 8 cores)
N_CORES = 8
CHUNK = 512         # out-features per matmul chunk
GRP = 4             # k-tiles quantized per pass
MAGIC = float(np.float32(1.5 * 2**23))
KT = K // P         # 32 k-tiles
MT = TOK // P       # 8 token tiles
NCHUNK = OUT // CHUNK  # 8 chunks

_cache = {}


def _build(mmdt):
    f32 = mybir.dt.float32
    nc = bacc.Bacc(None, target_bir_lowering=False)
    x_in = nc.declare_dram_parameter("x", [K, TOK], mmdt, isOutput=False)
    wT_in = nc.declare_dram_parameter("wT", [K, OUT], f32, isOutput=False)
    scale_in = nc.declare_dram_parameter("scale_row", [OUT], f32, isOutput=False)
    inv_in = nc.declare_dram_parameter("inv_row", [OUT], f32, isOutput=False)
    bdeq_in = nc.declare_dram_parameter("bdeq_row", [OUT], f32, isOutput=False)
    y_out = nc.declare_dram_parameter("y", [TOK, OUT], f32, isOutput=True)

    with tile.TileContext(nc) as tc:
        with tc.tile_pool(name="const", bufs=1) as const, \
             tc.tile_pool(name="xTp", bufs=1) as xTp, \
             tc.tile_pool(name="wTp", bufs=2) as wTp, \
             tc.tile_pool(name="stage", bufs=3) as stage, \
             tc.tile_pool(name="outp", bufs=4) as outp, \
             tc.tile_pool(name="mmps", bufs=6, space="PSUM") as mmps:

            ones_row = const.tile([1, P], mmdt)
            nc.vector.memset(ones_row[:], 1.0)

            brow = const.tile([1, OUT], mmdt)
            # SWDGE casts f32 -> bf16 during the transfer
            nc.gpsimd.dma_start(out=brow[:], in_=bdeq_in[:])

            def bcast_row(dram_param):
                a = dram_param[:]
                return bass.AP(tensor=a.tensor, offset=a.offset,
                               ap=[[0, P]] + list(a.ap))

            def bcast_row_slice(dram_param, lo, hi):
                a = dram_param[lo:hi]
                return bass.AP(tensor=a.tensor, offset=a.offset,
                               ap=[[0, P]] + list(a.ap))

            scale_bc = const.tile([P, OUT], f32)
            inv_bc = const.tile([P, OUT], f32)
            # HWDGE, split into slices so the first quant group only waits
            # for its own slice
            for c in range(NCHUNK):
                lo, hi = c * CHUNK, (c + 1) * CHUNK
                nc.sync.dma_start(out=inv_bc[:, lo:hi],
                                  in_=bcast_row_slice(inv_in, lo, hi))
                nc.sync.dma_start(out=scale_bc[:, lo:hi],
                                  in_=bcast_row_slice(scale_in, lo, hi))

            # ---- resident pre-transposed x ----
            xT = xTp.tile([P, KT * TOK], mmdt, name="xT")  # [:, kt*TOK + t]
            xT3 = xT[:].rearrange("p (kt t) -> p kt t", kt=KT)
            for kt in range(KT):
                nc.sync.dma_start(
                    out=xT3[:, kt, :], in_=x_in[kt * P:(kt + 1) * P, :])

            wT3d = wT_in[:].rearrange("(kt p) o -> p kt o", p=P)

            wTc3s = {}

            def quant_chunk(c):
                csl = slice(c * CHUNK, (c + 1) * CHUNK)
                wTc = wTp.tile([P, KT * CHUNK], mmdt, tag="wT")
                wTc3 = wTc[:].rearrange("p (kt t) -> p kt t", kt=KT)
                wTc3s[c] = wTc3
                for g in range(KT // GRP):
                    s = stage.tile([P, GRP * CHUNK], f32, tag="stage")
                    s3 = s[:].rearrange("p (i t) -> p i t", i=GRP)
                    nc.sync.dma_start(
                        out=s3, in_=wT3d[:, g * GRP:(g + 1) * GRP, csl])
                    inv_b = inv_bc[:, csl].unsqueeze(1).broadcast_to(
                        (P, GRP, CHUNK))
                    sc_b = scale_bc[:, csl].unsqueeze(1).broadcast_to(
                        (P, GRP, CHUNK))
                    nc.vector.tensor_tensor(
                        out=s3, in0=s3, in1=inv_b, op=mybir.AluOpType.mult)
                    nc.scalar.activation(
                        s3, s3, mybir.ActivationFunctionType.Copy, bias=MAGIC)
                    nc.vector.scalar_tensor_tensor(
                        out=wTc3[:, g * GRP:(g + 1) * GRP, :], in0=s3,
                        scalar=MAGIC, in1=sc_b,
                        op0=mybir.AluOpType.subtract, op1=mybir.AluOpType.mult)

            def matmul_chunk(c):
                csl = slice(c * CHUNK, (c + 1) * CHUNK)
                wTc3 = wTc3s.pop(c)
                for mt in range(MT):
                    ps = mmps.tile([P, CHUNK], f32, tag="mm")
                    nc.tensor.matmul(
                        ps[:], ones_row[:], brow[:, csl], start=True, stop=False)
                    for kt in range(KT):
                        nc.tensor.matmul(
                            ps[:],
                            xT3[:, kt, mt * P:(mt + 1) * P],
                            wTc3[:, kt, :],
                            start=False, stop=(kt == KT - 1))
                    ysb = outp.tile([P, CHUNK], f32, tag="ysb")
                    nc.scalar.activation(
                        ysb[:], ps[:], mybir.ActivationFunctionType.Copy)
                    nc.sync.dma_start(
                        out=y_out[mt * P:(mt + 1) * P, csl], in_=ysb[:])

            for c in range(NCHUNK + 1):
                if c < NCHUNK:
                    quant_chunk(c)
                if c >= 1:
                    matmul_chunk(c - 1)
    nc.compile()
    return nc


def _get_nc(mmdt):
    key = str(mmdt)
    if key not in _cache:
        _cache[key] = _build(mmdt)
    return _cache[key]


def _host_prep(x, weight, bias_param):
    B, S, _K = x.shape
    xb = np.asarray(x, dtype=np.float32).reshape(B * S, K).astype(ml_dtypes.bfloat16)
    w = np.asarray(weight, dtype=np.float32)
    b = np.asarray(bias_param, dtype=np.float32)

    # exact-f32 per-channel quant metadata (matches the jax reference ops)
    absmax = np.max(np.abs(w), axis=1)
    scale = (np.maximum(absmax, np.float32(2e-16)) / np.float32(7.0)).astype(np.float32)
    inv = (np.float32(1.0) / scale).astype(np.float32)
    bdeq = (np.round(b / scale) * scale).astype(np.float32)

    wT = np.ascontiguousarray(w.T)
    shards = [np.ascontiguousarray(xb[i * TOK:(i + 1) * TOK].T)
              for i in range(N_CORES)]
    return shards, wT, scale, inv, bdeq


def kernel(x: np.ndarray, weight: np.ndarray, bias_param: np.ndarray) -> np.ndarray:
    B, S, _K = x.shape
    assert (B * S, _K) == (TOK * N_CORES, K), (x.shape,)
    nc = _get_nc(mybir.dt.bfloat16)

    shards, wT, scale, inv, bdeq = _host_prep(x, weight, bias_param)
    in_maps = [
        {"x": shards[i], "wT": wT, "scale_row": scale,
         "inv_row": inv, "bdeq_row": bdeq}
        for i in range(N_CORES)
    ]
    trace = os.environ.get("BRW_TRACE", "0") == "1"
    res = run_bass_kernel_spmd(
        nc, in_maps, core_ids=list(range(N_CORES)), trace=trace)
    if trace:
        print(f"HW exec time: {res.exec_time_ns} ns", flush=True)
        kernel.last_exec_time_ns = res.exec_time_ns
        kernel.last_trace = res.instructions_and_trace
    y = np.concatenate([res.results[i]["y"] for i in range(N_CORES)], axis=0)
    return y.reshape(B, S, OUT)
